# revision 14
# baseline (speedup 1.0000x reference)
#
# nn_ExpHydroM100 kernel for 8 trn2 NeuronCores — fully on-device.
#
# Sharding: data-parallel over the basin axis B (8 basins per core,
# replicated MLP weights), per the sharding hint. Each core:
#   phase 0 (per time-segment): tier prep (midpoint forcings,
#     step(-temp), ln(lday)) on freshly DMA'd forcing slices
#   phase 1: the 2047-step RK4 scan (sequential in time), 8 segments of
#     256 steps, each a For_i hardware loop (5 steps unrolled per iter);
#     the y trajectory spills to a DRAM scratch tensor per segment
#   phase 2: final MLP pass over all [8, 2048] grid points, streaming
#     (p,t) and y back from DRAM in 512-sample chunks
# No collectives; the host only reshapes inputs/outputs.
#
# Layout notes: SBUF APs must start at partition 0/32/64/96 (PSUM is
# exempt) and every SBUF tensor's bytes are charged against all 128
# partitions (~208 KiB budget). So per-segment quantities live as
# free-dim BLOCKS of two small mega-tensors ([2, *] and [1, *]), forcing
# rows (p,t) and live state rows (s0,s1) are separate blocks with layer 1
# split into two PSUM-accumulating matmuls, and the output nonlinearity
# (relu(sinh(x)) == sinh(relu(x)) = (e^u - e^-u)/2, u = relu(o + b4)) is
# assembled in a PSUM scratch tile where partition offsets are legal.
#
import numpy as np

B, T, H = 64, 2048, 64
NCORES = 8
BPC = B // NCORES          # 8 basins per core
NS = T * BPC               # 16384 samples per core
WC = 350                   # packed weight columns
FD = 512                   # phase-2 free-dim chunk (one PSUM bank of fp32)
UNROLL = 5
SEGLEN = 256               # scan steps per segment
NSEG = (T - 1 + SEGLEN - 1) // SEGLEN
L8 = (SEGLEN + 1) * BPC    # seg cols incl. +1 step lookahead

_compiled = None


def _build_device():
    import concourse.bass as bass
    import concourse.bacc as bacc
    import concourse.mybir as mybir
    from concourse.tile import TileContext

    dt = mybir.dt.float32
    AF = mybir.ActivationFunctionType
    OP = mybir.AluOpType

    nc = bacc.Bacc(None, target_bir_lowering=False)
    ptr = nc.declare_dram_parameter("ptr", [2, NS], dt, isOutput=False)
    trw = nc.declare_dram_parameter("trw", [1, NS], dt, isOutput=False)
    ldr = nc.declare_dram_parameter("ldr", [1, NS], dt, isOutput=False)
    y0d = nc.declare_dram_parameter("y0d", [2, BPC], dt, isOutput=False)
    wpk = nc.declare_dram_parameter("wpk", [H, WC], dt, isOutput=False)
    qout = nc.declare_dram_parameter("q", [1, NS], dt, isOutput=True)
    ydram = nc.dram_tensor("yscratch", [2, NS], dt, kind="Internal")

    with TileContext(nc) as tc:
        with (
            tc.tile_pool(name="pers", bufs=1) as pers,
            tc.tile_pool(name="wk", bufs=3) as wk,
            tc.tile_pool(name="ps", bufs=2, space="PSUM") as ps,
            tc.tile_pool(name="ps2", bufs=4, space="PSUM") as ps2,
        ):
            # mega2 blocks ([2, L8] each): 0 PT | 1 PTM | 2 YX | 3 YM1
            #   | 4 YM2 | 5 YE
            # mega1 blocks ([1, L8] each): 0 SNT | 1 SNTM | 2 LNX | 3 LNXM
            #   | 4 TRW | 5 LD | 6 S1
            mega2 = pers.tile([2, 6 * L8], dt, tag="mega2")
            mega1 = pers.tile([1, 7 * L8], dt, tag="mega1")
            tw = pers.tile([H, WC], dt, tag="tw")

            def m2(b, c):  # [2, 8] slice of mega2 block b at seg-col c
                if isinstance(c, int):
                    return mega2[0:2, b * L8 + c: b * L8 + c + BPC]
                return mega2[0:2, bass.ds(b * L8 + c, BPC)]

            def m1(b, c):
                if isinstance(c, int):
                    return mega1[0:1, b * L8 + c: b * L8 + c + BPC]
                return mega1[0:1, bass.ds(b * L8 + c, BPC)]

            def blk2(b, c0, n):  # [2, n] block slice (static)
                return mega2[0:2, b * L8 + c0: b * L8 + c0 + n]

            def blk1(b, c0, n):
                return mega1[0:1, b * L8 + c0: b * L8 + c0 + n]

            nc.sync.dma_start(tw[:], wpk[:])

            W2 = tw[:, 0:64]
            W3 = tw[:, 64:128]
            W4q = tw[:, 128:129]
            W4u = tw[:, 129:132]
            W4e = tw[:, 132:134]
            b1 = tw[:, 134:135]
            b2 = tw[:, 135:136]
            b3 = tw[:, 136:137]
            bq = tw[0:1, 137:138]
            W1y = tw[0:2, 138:202]
            W1pt = tw[0:2, 202:266]
            W1ptm = tw[0:2, 266:330]
            oh2 = tw[0:1, 330:332]
            sel_sn = tw[0:1, 332:335]
            sel_g = tw[0:2, 335:338]
            sel_q = tw[0:2, 338:340]
            Ma3 = tw[0:3, 340:342]
            Mb3 = tw[0:3, 342:344]
            Me2 = tw[0:2, 344:346]
            Mbe2 = tw[0:2, 346:348]
            b4a = tw[0:3, 348:349]
            b4e = tw[0:2, 349:350]

            # All engine APs (SBUF and PSUM) must start at partition
            # 0/32/64/96 (matmul OUTPUTS only 0/32/64), so small PSUM tiles
            # share one bank at quadrant offsets: 0:3 pou (o0..o2)
            #   | 32:35 facp [snt,snt,gt0] | 64:66 facq [gt1,gt1];
            # pod (o3+ln(lday), o4) gets its own tile.
            def stage(ptb, ptc, W1p, yb, yc, snb, snc, lnb, lnc):
                ph1 = ps.tile([H, BPC], dt, tag="ph")
                h1 = wk.tile([H, BPC], dt, tag="h1")
                nc.tensor.matmul(ph1[:], W1p, m2(ptb, ptc), start=True, stop=False)
                nc.tensor.matmul(ph1[:], W1y, m2(yb, yc), start=False, stop=True)
                nc.scalar.activation(h1[:], ph1[:], AF.Tanh, bias=b1)
                ph2 = ps.tile([H, BPC], dt, tag="ph")
                h2 = wk.tile([H, BPC], dt, tag="h2")
                nc.tensor.matmul(ph2[:], W2, h1[:], start=True, stop=True)
                nc.scalar.activation(h2[:], ph2[:], AF.Tanh, bias=b2)
                ph3 = ps.tile([H, BPC], dt, tag="ph")
                h3 = wk.tile([H, BPC], dt, tag="h3")
                nc.tensor.matmul(ph3[:], W3, h2[:], start=True, stop=True)
                nc.scalar.activation(h3[:], ph3[:], AF.Tanh, bias=b3)
                pou = ps2.tile([3, BPC], dt, tag="pou", bufs=1)
                pod = ps2.tile([2, BPC], dt, tag="pod", bufs=1)
                facp = ps2.tile([3, BPC], dt, tag="facp", bufs=1)
                facq = ps2.tile([2, BPC], dt, tag="facq", bufs=1)
                pou, pod, facp, facq = pou[:], pod[:], facp[:], facq[:]
                gt2 = wk.tile([2, BPC], dt, tag="gt2")
                nc.scalar.activation(gt2[:], m2(yb, yc), AF.Tanh, scale=5.0)
                nc.tensor.matmul(pod, oh2, m1(lnb, lnc), start=True, stop=False)
                nc.tensor.matmul(pod, W4e, h3[:], start=False, stop=True)
                nc.tensor.matmul(pou, W4u, h3[:], start=True, stop=True)
                nc.tensor.matmul(facp, sel_sn, m1(snb, snc), start=True, stop=False)
                nc.tensor.matmul(facp, sel_g, gt2[:], start=False, stop=True)
                nc.tensor.matmul(facq, sel_q, gt2[:], start=True, stop=True)
                u = wk.tile([3, BPC], dt, tag="u")
                ep = wk.tile([3, BPC], dt, tag="ep")
                en = wk.tile([3, BPC], dt, tag="en")
                sh3 = wk.tile([3, BPC], dt, tag="sh3")
                ee2 = wk.tile([2, BPC], dt, tag="ee2")
                shm3 = wk.tile([3, BPC], dt, tag="shm3")
                eem2 = wk.tile([2, BPC], dt, tag="eem2")
                nc.scalar.activation(u[:], pou, AF.Relu, bias=b4a)
                nc.scalar.activation(ep[:], u[:], AF.Exp)
                nc.scalar.activation(en[:], u[:], AF.Exp, scale=-1.0)
                nc.scalar.activation(ee2[:], pod, AF.Exp, bias=b4e)
                nc.vector.tensor_sub(sh3[:], ep[:], en[:])
                nc.vector.tensor_mul(shm3[:], sh3[:], facp)
                nc.vector.tensor_mul(eem2[:], ee2[:], facq)
                kst = ps2.tile([2, BPC], dt, tag="kst", bufs=2)
                nc.tensor.matmul(kst[:], Ma3, sh3[:], start=True, stop=False)
                nc.tensor.matmul(kst[:], Mb3, shm3[:], start=False, stop=False)
                nc.tensor.matmul(kst[:], Me2, ee2[:], start=False, stop=False)
                nc.tensor.matmul(kst[:], Mbe2, eem2[:], start=False, stop=True)
                return kst

            def step(ci, cip1):
                y = m2(2, ci)
                k1 = stage(0, ci, W1pt, 2, ci, 0, ci, 2, ci)[:]
                nc.vector.scalar_tensor_tensor(
                    m2(3, ci), k1, 0.5, y, OP.mult, OP.add)
                u1 = wk.tile([2, BPC], dt, tag="uc")
                nc.vector.scalar_tensor_tensor(
                    u1[:], k1, 1.0 / 6.0, y, OP.mult, OP.add)
                k2 = stage(1, ci, W1ptm, 3, ci, 1, ci, 3, ci)[:]
                nc.vector.scalar_tensor_tensor(
                    m2(4, ci), k2, 0.5, y, OP.mult, OP.add)
                u2 = wk.tile([2, BPC], dt, tag="uc")
                nc.vector.scalar_tensor_tensor(
                    u2[:], k2, 1.0 / 3.0, u1[:], OP.mult, OP.add)
                k3 = stage(1, ci, W1ptm, 4, ci, 1, ci, 3, ci)[:]
                nc.vector.tensor_add(m2(5, ci), k3, y)
                u3 = wk.tile([2, BPC], dt, tag="uc")
                nc.vector.scalar_tensor_tensor(
                    u3[:], k3, 1.0 / 3.0, u2[:], OP.mult, OP.add)
                k4 = stage(0, cip1, W1pt, 5, ci, 0, cip1, 2, cip1)[:]
                nc.vector.scalar_tensor_tensor(
                    m2(2, cip1), k4, 1.0 / 6.0, u3[:], OP.mult, OP.add)

            for s in range(NSEG):
                seg_start = s * SEGLEN
                nst = min(SEGLEN, (T - 1) - seg_start)
                c8 = seg_start * BPC
                ncols = nst * BPC + BPC  # forcing cols incl. lookahead

                # ---- per-segment phase 0 ----
                nc.sync.dma_start(blk2(0, 0, ncols), ptr[0:2, c8:c8 + ncols])
                nc.sync.dma_start(blk1(4, 0, ncols), trw[0:1, c8:c8 + ncols])
                nc.sync.dma_start(blk1(5, 0, ncols), ldr[0:1, c8:c8 + ncols])
                if s == 0:
                    nc.sync.dma_start(mega2[0:2, 2 * L8:2 * L8 + BPC], y0d[:])
                else:
                    nc.vector.tensor_copy(m2(2, 0), m2(2, SEGLEN * BPC))
                nm = nst * BPC
                nc.vector.tensor_add(blk2(1, 0, nm), blk2(0, 0, nm),
                                     blk2(0, BPC, nm))
                nc.vector.tensor_add(blk1(6, 0, nm), blk1(4, 0, nm),
                                     blk1(4, BPC, nm))
                nc.scalar.activation(blk1(1, 0, nm), blk1(6, 0, nm),
                                     AF.Tanh, scale=-2.5)
                nc.scalar.activation(blk1(0, 0, ncols), blk1(4, 0, ncols),
                                     AF.Tanh, scale=-5.0)
                nc.vector.tensor_add(blk1(6, 0, nm), blk1(5, 0, nm),
                                     blk1(5, BPC, nm))
                nc.scalar.activation(blk1(3, 0, nm), blk1(6, 0, nm),
                                     AF.Ln, scale=0.5)
                nc.scalar.activation(blk1(2, 0, ncols), blk1(5, 0, ncols),
                                     AF.Ln)

                # ---- scan over this segment ----
                nit = nst // UNROLL
                with tc.For_i(0, nit) as it:
                    base = it * (BPC * UNROLL)
                    for j in range(UNROLL):
                        step(base + BPC * j, base + BPC * (j + 1))
                for i in range(nit * UNROLL, nst):
                    step(i * BPC, (i + 1) * BPC)

                # spill y trajectory for this segment's steps
                nc.sync.dma_start(ydram[0:2, c8:c8 + nm],
                                  blk2(2, 0, nm))
            # final state y(T-1) lives at seg col nst*BPC of the last segment
            lc8 = (NSEG - 1) * SEGLEN * BPC
            lnm = ((T - 1) - (NSEG - 1) * SEGLEN) * BPC
            nc.sync.dma_start(ydram[0:2, lc8 + lnm:lc8 + lnm + BPC],
                              blk2(2, lnm, BPC))

            # ---- phase 2: final MLP pass over all NS samples ----
            for ch in range(NS // FD):
                sl = slice(ch * FD, (ch + 1) * FD)
                p2pt = wk.tile([2, FD], dt, tag="p2pt")
                p2y = wk.tile([2, FD], dt, tag="p2y")
                nc.sync.dma_start(p2pt[:], ptr[0:2, sl])
                nc.sync.dma_start(p2y[:], ydram[0:2, sl])
                pa = ps.tile([H, FD], dt, tag="ph")
                ha = wk.tile([H, FD], dt, tag="h1")
                nc.tensor.matmul(pa[:], W1pt, p2pt[:], start=True, stop=False)
                nc.tensor.matmul(pa[:], W1y, p2y[:], start=False, stop=True)
                nc.scalar.activation(ha[:], pa[:], AF.Tanh, bias=b1)
                pb = ps.tile([H, FD], dt, tag="ph")
                hb = wk.tile([H, FD], dt, tag="h2")
                nc.tensor.matmul(pb[:], W2, ha[:], start=True, stop=True)
                nc.scalar.activation(hb[:], pb[:], AF.Tanh, bias=b2)
                pc = ps.tile([H, FD], dt, tag="ph")
                hc = wk.tile([H, FD], dt, tag="h3")
                nc.tensor.matmul(pc[:], W3, hb[:], start=True, stop=True)
                nc.scalar.activation(hc[:], pc[:], AF.Tanh, bias=b3)
                pq = ps2.tile([1, FD], dt, tag="pod", bufs=1)
                qch = wk.tile([1, FD], dt, tag="qch")
                nc.tensor.matmul(pq[:], W4q, hc[:], start=True, stop=True)
                nc.vector.tensor_scalar_add(qch[:], pq[:], bq)
                nc.sync.dma_start(qout[0:1, sl], qch[:])

    nc.compile()
    return nc


def _pack_weights(W1, b1, W2, b2, W3, b3, W4, b4):
    f32 = np.float32
    wpk = np.zeros((H, WC), f32)
    wpk[:, 0:64] = W2
    wpk[:, 64:128] = W3
    wpk[:, 128] = W4[:, 4]
    wpk[:, 129:134] = W4
    wpk[:, 134] = b1
    wpk[:, 135] = b2
    wpk[:, 136] = b3
    wpk[0, 137] = b4[4]
    wpk[0:2, 138:202] = W1[0:2]
    wpk[0:2, 202:266] = W1[2:4]
    wpk[0:2, 266:330] = 0.5 * W1[2:4]
    wpk[0, 330:332] = np.array([1, 0], f32)            # oh2
    wpk[0, 332:335] = np.array([1, 1, 0], f32)          # sel_sn
    wpk[0:2, 335:338] = np.array([[0, 0, 1], [0, 0, 0]], f32)  # sel_g
    wpk[0:2, 338:340] = np.array([[0, 0], [1, 1]], f32)        # sel_q
    wpk[0:3, 340:342] = np.array([[.25, 0], [0, .5], [-.25, .25]], f32)  # Ma3
    wpk[0:3, 342:344] = np.array([[.25, 0], [0, 0], [-.25, .25]], f32)   # Mb3
    wpk[0:2, 344:346] = np.array([[0, -.5], [0, -.5]], f32)    # Me2
    wpk[0:2, 346:348] = np.array([[0, -.5], [0, -.5]], f32)    # Mbe2
    wpk[0:3, 348] = b4[0:3]
    wpk[0:2, 349] = b4[3:5]
    return wpk


def kernel(s_snow, s_water, precp_series, tmean_series, lday_series, time_series,
           W1, b1, W2, b2, W3, b3, W4, b4):
    global _compiled
    f32 = np.float32
    args = [np.asarray(a, f32) for a in
            (s_snow, s_water, precp_series, tmean_series, lday_series,
             time_series, W1, b1, W2, b2, W3, b3, W4, b4)]
    (s_snow, s_water, precp, tmean, lday, tser,
     W1, b1, W2, b2, W3, b3, W4, b4) = args

    from concourse.bass_utils import run_bass_kernel_spmd
    if _compiled is None:
        _compiled = _build_device()
    nc = _compiled

    wpk = _pack_weights(W1, b1, W2, b2, W3, b3, W4, b4)
    in_maps = []
    for c in range(NCORES):
        bs = slice(c * BPC, (c + 1) * BPC)
        pt = np.empty((2, NS), f32)
        pt[0] = np.ascontiguousarray(precp[bs].T).reshape(NS)
        pt[1] = np.ascontiguousarray(tmean[bs].T).reshape(NS)
        in_maps.append({
            "ptr": pt,
            "trw": pt[1:2].copy(),
            "ldr": np.ascontiguousarray(lday[bs].T).reshape(1, NS),
            "y0d": np.stack([s_snow[bs, 0], s_water[bs, 0]]).copy(),
            "wpk": wpk,
        })
    res = run_bass_kernel_spmd(nc, in_maps, list(range(NCORES)))

    q = np.empty((B, T), f32)
    for c in range(NCORES):
        qc = np.asarray(res.results[c]["q"]).reshape(T, BPC)
        q[c * BPC:(c + 1) * BPC, :] = qc.T
    return q


# revision 15
# speedup vs baseline: 5.3844x; 5.3844x over previous
#
# nn_ExpHydroM100 kernel for 8 trn2 NeuronCores — fully on-device.
#
# Sharding: data-parallel over the basin axis B (8 basins per core,
# replicated MLP weights), per the sharding hint. Each core:
#   phase 0 (per time-segment): tier prep (midpoint forcings,
#     step(-temp), ln(lday)) on freshly DMA'd forcing slices
#   phase 1: the 2047-step RK4 scan (sequential in time), 8 segments of
#     256 steps, each a For_i hardware loop (5 steps unrolled per iter);
#     the y trajectory spills to a DRAM scratch tensor per segment
#   phase 2: final MLP pass over all [8, 2048] grid points, streaming
#     (p,t) and y back from DRAM in 512-sample chunks
# No collectives; the host only reshapes inputs/outputs.
#
# Layout notes: SBUF APs must start at partition 0/32/64/96 (PSUM is
# exempt) and every SBUF tensor's bytes are charged against all 128
# partitions (~208 KiB budget). So per-segment quantities live as
# free-dim BLOCKS of two small mega-tensors ([2, *] and [1, *]), forcing
# rows (p,t) and live state rows (s0,s1) are separate blocks with layer 1
# split into two PSUM-accumulating matmuls, and the output nonlinearity
# (relu(sinh(x)) == sinh(relu(x)) = (e^u - e^-u)/2, u = relu(o + b4)) is
# assembled in a PSUM scratch tile where partition offsets are legal.
#
import numpy as np

B, T, H = 64, 2048, 64
NCORES = 8
BPC = B // NCORES          # 8 basins per core
NS = T * BPC               # 16384 samples per core
WC = 350                   # packed weight columns
FD = 512                   # phase-2 free-dim chunk (one PSUM bank of fp32)
UNROLL = 5
SEGLEN = 256               # scan steps per segment
NSEG = (T - 1 + SEGLEN - 1) // SEGLEN
L8 = (SEGLEN + 1) * BPC    # seg cols incl. +1 step lookahead

_compiled = None


def _build_device():
    import concourse.bass as bass
    import concourse.bacc as bacc
    import concourse.mybir as mybir
    from concourse.tile import TileContext

    dt = mybir.dt.float32
    AF = mybir.ActivationFunctionType
    OP = mybir.AluOpType

    nc = bacc.Bacc(None, target_bir_lowering=False)
    ptr = nc.declare_dram_parameter("ptr", [2, NS], dt, isOutput=False)
    trw = nc.declare_dram_parameter("trw", [1, NS], dt, isOutput=False)
    ldr = nc.declare_dram_parameter("ldr", [1, NS], dt, isOutput=False)
    y0d = nc.declare_dram_parameter("y0d", [2, BPC], dt, isOutput=False)
    wpk = nc.declare_dram_parameter("wpk", [H, WC], dt, isOutput=False)
    qout = nc.declare_dram_parameter("q", [1, NS], dt, isOutput=True)
    ydram = nc.dram_tensor("yscratch", [2, NS], dt, kind="Internal")

    with TileContext(nc) as tc:
        with (
            tc.tile_pool(name="pers", bufs=1) as pers,
            tc.tile_pool(name="wk", bufs=3) as wk,
            tc.tile_pool(name="ps", bufs=2, space="PSUM") as ps,
            tc.tile_pool(name="ps2", bufs=4, space="PSUM") as ps2,
        ):
            # mega2 blocks ([2, L8] each): 0 PT | 1 PTM | 2 YX | 3 YM1
            #   | 4 YM2 | 5 YE
            # mega1 blocks ([1, L8] each): 0 SNT | 1 SNTM | 2 LNX | 3 LNXM
            #   | 4 TRW | 5 LD | 6 S1
            mega2 = pers.tile([2, 6 * L8], dt, tag="mega2")
            mega1 = pers.tile([1, 7 * L8], dt, tag="mega1")
            tw = pers.tile([H, WC], dt, tag="tw")

            def m2(b, c):  # [2, 8] slice of mega2 block b at seg-col c
                if isinstance(c, int):
                    return mega2[0:2, b * L8 + c: b * L8 + c + BPC]
                return mega2[0:2, bass.ds(b * L8 + c, BPC)]

            def m1(b, c):
                if isinstance(c, int):
                    return mega1[0:1, b * L8 + c: b * L8 + c + BPC]
                return mega1[0:1, bass.ds(b * L8 + c, BPC)]

            def blk2(b, c0, n):  # [2, n] block slice (static)
                return mega2[0:2, b * L8 + c0: b * L8 + c0 + n]

            def blk1(b, c0, n):
                return mega1[0:1, b * L8 + c0: b * L8 + c0 + n]

            nc.sync.dma_start(tw[:], wpk[:])

            W2 = tw[:, 0:64]
            W3 = tw[:, 64:128]
            W4q = tw[:, 128:129]
            W4u = tw[:, 129:132]
            W4e = tw[:, 132:134]
            b1 = tw[:, 134:135]
            b2 = tw[:, 135:136]
            b3 = tw[:, 136:137]
            bq = tw[0:1, 137:138]
            W1y = tw[0:2, 138:202]
            W1pt = tw[0:2, 202:266]
            W1ptm = tw[0:2, 266:330]
            oh2 = tw[0:1, 330:332]
            sel_sn = tw[0:1, 332:335]
            sel_g = tw[0:2, 335:338]
            sel_q = tw[0:2, 338:340]
            Ma3 = tw[0:3, 340:342]
            Mb3 = tw[0:3, 342:344]
            Me2 = tw[0:2, 344:346]
            Mbe2 = tw[0:2, 346:348]
            b4a = tw[0:3, 348:349]
            b4e = tw[0:2, 349:350]

            # All engine APs (SBUF and PSUM) must start at partition
            # 0/32/64/96 (matmul OUTPUTS only 0/32/64), so small PSUM tiles
            # share one bank at quadrant offsets: 0:3 pou (o0..o2)
            #   | 32:35 facp [snt,snt,gt0] | 64:66 facq [gt1,gt1];
            # pod (o3+ln(lday), o4) gets its own tile.
            def stage(ptb, ptc, W1p, yb, yc, snb, snc, lnb, lnc):
                ph1 = ps.tile([H, BPC], dt, tag="ph")
                h1 = wk.tile([H, BPC], dt, tag="h1")
                nc.tensor.matmul(ph1[:], W1p, m2(ptb, ptc), start=True, stop=False)
                nc.tensor.matmul(ph1[:], W1y, m2(yb, yc), start=False, stop=True)
                nc.scalar.activation(h1[:], ph1[:], AF.Tanh, bias=b1)
                ph2 = ps.tile([H, BPC], dt, tag="ph")
                h2 = wk.tile([H, BPC], dt, tag="h2")
                nc.tensor.matmul(ph2[:], W2, h1[:], start=True, stop=True)
                nc.scalar.activation(h2[:], ph2[:], AF.Tanh, bias=b2)
                ph3 = ps.tile([H, BPC], dt, tag="ph")
                h3 = wk.tile([H, BPC], dt, tag="h3")
                nc.tensor.matmul(ph3[:], W3, h2[:], start=True, stop=True)
                nc.scalar.activation(h3[:], ph3[:], AF.Tanh, bias=b3)
                pou = ps2.tile([3, BPC], dt, tag="pou", bufs=1)
                pod = ps2.tile([2, BPC], dt, tag="pod", bufs=1)
                facp = ps2.tile([3, BPC], dt, tag="facp", bufs=1)
                facq = ps2.tile([2, BPC], dt, tag="facq", bufs=1)
                pou, pod, facp, facq = pou[:], pod[:], facp[:], facq[:]
                gt2 = wk.tile([2, BPC], dt, tag="gt2")
                nc.scalar.activation(gt2[:], m2(yb, yc), AF.Tanh, scale=5.0)
                nc.tensor.matmul(pod, oh2, m1(lnb, lnc), start=True, stop=False)
                nc.tensor.matmul(pod, W4e, h3[:], start=False, stop=True)
                nc.tensor.matmul(pou, W4u, h3[:], start=True, stop=True)
                nc.tensor.matmul(facp, sel_sn, m1(snb, snc), start=True, stop=False)
                nc.tensor.matmul(facp, sel_g, gt2[:], start=False, stop=True)
                nc.tensor.matmul(facq, sel_q, gt2[:], start=True, stop=True)
                u = wk.tile([3, BPC], dt, tag="u")
                ep = wk.tile([3, BPC], dt, tag="ep")
                en = wk.tile([3, BPC], dt, tag="en")
                sh3 = wk.tile([3, BPC], dt, tag="sh3")
                ee2 = wk.tile([2, BPC], dt, tag="ee2")
                shm3 = wk.tile([3, BPC], dt, tag="shm3")
                eem2 = wk.tile([2, BPC], dt, tag="eem2")
                nc.scalar.activation(u[:], pou, AF.Relu, bias=b4a)
                nc.scalar.activation(ep[:], u[:], AF.Exp)
                nc.scalar.activation(en[:], u[:], AF.Exp, scale=-1.0)
                nc.scalar.activation(ee2[:], pod, AF.Exp, bias=b4e)
                nc.vector.tensor_sub(sh3[:], ep[:], en[:])
                nc.vector.tensor_mul(shm3[:], sh3[:], facp)
                nc.vector.tensor_mul(eem2[:], ee2[:], facq)
                kst = ps2.tile([2, BPC], dt, tag="kst", bufs=2)
                nc.tensor.matmul(kst[:], Ma3, sh3[:], start=True, stop=False)
                nc.tensor.matmul(kst[:], Mb3, shm3[:], start=False, stop=False)
                nc.tensor.matmul(kst[:], Me2, ee2[:], start=False, stop=False)
                nc.tensor.matmul(kst[:], Mbe2, eem2[:], start=False, stop=True)
                return kst

            def step(ci, cip1):
                y = m2(2, ci)
                k1 = stage(0, ci, W1pt, 2, ci, 0, ci, 2, ci)[:]
                nc.vector.scalar_tensor_tensor(
                    m2(3, ci), k1, 0.5, y, OP.mult, OP.add)
                u1 = wk.tile([2, BPC], dt, tag="uc")
                nc.vector.scalar_tensor_tensor(
                    u1[:], k1, 1.0 / 6.0, y, OP.mult, OP.add)
                k2 = stage(1, ci, W1ptm, 3, ci, 1, ci, 3, ci)[:]
                nc.vector.scalar_tensor_tensor(
                    m2(4, ci), k2, 0.5, y, OP.mult, OP.add)
                u2 = wk.tile([2, BPC], dt, tag="uc")
                nc.vector.scalar_tensor_tensor(
                    u2[:], k2, 1.0 / 3.0, u1[:], OP.mult, OP.add)
                k3 = stage(1, ci, W1ptm, 4, ci, 1, ci, 3, ci)[:]
                nc.vector.tensor_add(m2(5, ci), k3, y)
                u3 = wk.tile([2, BPC], dt, tag="uc")
                nc.vector.scalar_tensor_tensor(
                    u3[:], k3, 1.0 / 3.0, u2[:], OP.mult, OP.add)
                k4 = stage(0, cip1, W1pt, 5, ci, 0, cip1, 2, cip1)[:]
                nc.vector.scalar_tensor_tensor(
                    m2(2, cip1), k4, 1.0 / 6.0, u3[:], OP.mult, OP.add)

            for s in range(NSEG):
                seg_start = s * SEGLEN
                nst = min(SEGLEN, (T - 1) - seg_start)
                c8 = seg_start * BPC
                ncols = nst * BPC + BPC  # forcing cols incl. lookahead

                # ---- per-segment phase 0 ----
                nc.sync.dma_start(blk2(0, 0, ncols), ptr[0:2, c8:c8 + ncols])
                nc.sync.dma_start(blk1(4, 0, ncols), trw[0:1, c8:c8 + ncols])
                nc.sync.dma_start(blk1(5, 0, ncols), ldr[0:1, c8:c8 + ncols])
                if s == 0:
                    nc.sync.dma_start(mega2[0:2, 2 * L8:2 * L8 + BPC], y0d[:])
                else:
                    nc.vector.tensor_copy(m2(2, 0), m2(2, SEGLEN * BPC))
                nm = nst * BPC
                nc.vector.tensor_add(blk2(1, 0, nm), blk2(0, 0, nm),
                                     blk2(0, BPC, nm))
                nc.vector.tensor_add(blk1(6, 0, nm), blk1(4, 0, nm),
                                     blk1(4, BPC, nm))
                nc.scalar.activation(blk1(1, 0, nm), blk1(6, 0, nm),
                                     AF.Tanh, scale=-2.5)
                nc.scalar.activation(blk1(0, 0, ncols), blk1(4, 0, ncols),
                                     AF.Tanh, scale=-5.0)
                nc.vector.tensor_add(blk1(6, 0, nm), blk1(5, 0, nm),
                                     blk1(5, BPC, nm))
                nc.scalar.activation(blk1(3, 0, nm), blk1(6, 0, nm),
                                     AF.Ln, scale=0.5)
                nc.scalar.activation(blk1(2, 0, ncols), blk1(5, 0, ncols),
                                     AF.Ln)

                # ---- scan over this segment ----
                nit = nst // UNROLL
                with tc.For_i(0, nit) as it:
                    base = it * (BPC * UNROLL)
                    for j in range(UNROLL):
                        step(base + BPC * j, base + BPC * (j + 1))
                for i in range(nit * UNROLL, nst):
                    step(i * BPC, (i + 1) * BPC)

                # spill y trajectory for this segment's steps
                nc.sync.dma_start(ydram[0:2, c8:c8 + nm],
                                  blk2(2, 0, nm))
            # final state y(T-1) lives at seg col nst*BPC of the last segment
            lc8 = (NSEG - 1) * SEGLEN * BPC
            lnm = ((T - 1) - (NSEG - 1) * SEGLEN) * BPC
            nc.sync.dma_start(ydram[0:2, lc8 + lnm:lc8 + lnm + BPC],
                              blk2(2, lnm, BPC))

            # ---- phase 2: final MLP pass over all NS samples ----
            for ch in range(NS // FD):
                sl = slice(ch * FD, (ch + 1) * FD)
                p2pt = wk.tile([2, FD], dt, tag="p2pt")
                p2y = wk.tile([2, FD], dt, tag="p2y")
                nc.sync.dma_start(p2pt[:], ptr[0:2, sl])
                nc.sync.dma_start(p2y[:], ydram[0:2, sl])
                pa = ps.tile([H, FD], dt, tag="ph")
                ha = wk.tile([H, FD], dt, tag="h1")
                nc.tensor.matmul(pa[:], W1pt, p2pt[:], start=True, stop=False)
                nc.tensor.matmul(pa[:], W1y, p2y[:], start=False, stop=True)
                nc.scalar.activation(ha[:], pa[:], AF.Tanh, bias=b1)
                pb = ps.tile([H, FD], dt, tag="ph")
                hb = wk.tile([H, FD], dt, tag="h2")
                nc.tensor.matmul(pb[:], W2, ha[:], start=True, stop=True)
                nc.scalar.activation(hb[:], pb[:], AF.Tanh, bias=b2)
                pc = ps.tile([H, FD], dt, tag="ph")
                hc = wk.tile([H, FD], dt, tag="h3")
                nc.tensor.matmul(pc[:], W3, hb[:], start=True, stop=True)
                nc.scalar.activation(hc[:], pc[:], AF.Tanh, bias=b3)
                pq = ps2.tile([1, FD], dt, tag="pod", bufs=1)
                qch = wk.tile([1, FD], dt, tag="qch")
                nc.tensor.matmul(pq[:], W4q, hc[:], start=True, stop=True)
                nc.vector.tensor_scalar_add(qch[:], pq[:], bq)
                nc.sync.dma_start(qout[0:1, sl], qch[:])

    nc.compile()
    return nc


def _pack_weights(W1, b1, W2, b2, W3, b3, W4, b4):
    f32 = np.float32
    wpk = np.zeros((H, WC), f32)
    wpk[:, 0:64] = W2
    wpk[:, 64:128] = W3
    wpk[:, 128] = W4[:, 4]
    wpk[:, 129:134] = W4
    wpk[:, 134] = b1
    wpk[:, 135] = b2
    wpk[:, 136] = b3
    wpk[0, 137] = b4[4]
    wpk[0:2, 138:202] = W1[0:2]
    wpk[0:2, 202:266] = W1[2:4]
    wpk[0:2, 266:330] = 0.5 * W1[2:4]
    wpk[0, 330:332] = np.array([1, 0], f32)            # oh2
    wpk[0, 332:335] = np.array([1, 1, 0], f32)          # sel_sn
    wpk[0:2, 335:338] = np.array([[0, 0, 1], [0, 0, 0]], f32)  # sel_g
    wpk[0:2, 338:340] = np.array([[0, 0], [1, 1]], f32)        # sel_q
    wpk[0:3, 340:342] = np.array([[.25, 0], [0, .5], [-.25, .25]], f32)  # Ma3
    wpk[0:3, 342:344] = np.array([[.25, 0], [0, 0], [-.25, .25]], f32)   # Mb3
    wpk[0:2, 344:346] = np.array([[0, -.5], [0, -.5]], f32)    # Me2
    wpk[0:2, 346:348] = np.array([[0, -.5], [0, -.5]], f32)    # Mbe2
    wpk[0:3, 348] = b4[0:3]
    wpk[0:2, 349] = b4[3:5]
    return wpk


def _make_cached_runner(nc):
    """One-time jax.jit of the bass program (same _bass_exec_p lowering that
    run_bass_kernel_spmd uses under axon) so repeat calls skip re-tracing."""
    import jax
    import numpy as onp
    from jax.sharding import Mesh, PartitionSpec
    from jax.experimental.shard_map import shard_map
    import concourse.mybir as mybir
    from concourse.bass2jax import (
        install_neuronx_cc_hook, _bass_exec_p, partition_id_tensor)
    install_neuronx_cc_hook()

    partition_name = (nc.partition_id_tensor.name
                      if nc.partition_id_tensor else None)
    in_names, out_names, out_avals, zero_outs = [], [], [], []
    for alloc in nc.m.functions[0].allocations:
        if not isinstance(alloc, mybir.MemoryLocationSet):
            continue
        name = alloc.memorylocations[0].name
        if alloc.kind == "ExternalInput":
            if name != partition_name:
                in_names.append(name)
        elif alloc.kind == "ExternalOutput":
            shape = tuple(alloc.tensor_shape)
            dtype = mybir.dt.np(alloc.dtype)
            out_names.append(name)
            out_avals.append(jax.core.ShapedArray(shape, dtype))
            zero_outs.append(onp.zeros(shape, dtype))
    n_params, n_outs = len(in_names), len(out_avals)
    in_names_full = in_names + out_names + (
        [partition_name] if partition_name else [])
    donate = tuple(range(n_params, n_params + n_outs))

    def _body(*xs):
        ops = list(xs)
        if partition_name is not None:
            ops.append(partition_id_tensor())
        return tuple(_bass_exec_p.bind(
            *ops, out_avals=tuple(out_avals), in_names=tuple(in_names_full),
            out_names=tuple(out_names), lowering_input_output_aliases=(),
            sim_require_finite=True, sim_require_nnan=True, nc=nc))

    devices = jax.devices()[:NCORES]
    mesh = Mesh(onp.asarray(devices), ("core",))
    in_specs = (PartitionSpec("core"),) * (n_params + n_outs)
    out_specs = (PartitionSpec("core"),) * n_outs
    sharded = jax.jit(
        shard_map(_body, mesh=mesh, in_specs=in_specs,
                  out_specs=out_specs, check_rep=False),
        donate_argnums=donate, keep_unused=True)

    def run(in_maps):
        per_core = [[onp.asarray(m[nm]) for nm in in_names] for m in in_maps]
        concat_in = [
            onp.concatenate([per_core[c][i] for c in range(NCORES)], axis=0)
            for i in range(n_params)]
        concat_zeros = [
            onp.zeros((NCORES * z.shape[0], *z.shape[1:]), z.dtype)
            for z in zero_outs]
        outs = sharded(*concat_in, *concat_zeros)
        return [{nm: onp.asarray(outs[i]).reshape(
                    NCORES, *out_avals[i].shape)[c]
                 for i, nm in enumerate(out_names)}
                for c in range(NCORES)]

    return run


def kernel(s_snow, s_water, precp_series, tmean_series, lday_series, time_series,
           W1, b1, W2, b2, W3, b3, W4, b4):
    global _compiled
    f32 = np.float32
    args = [np.asarray(a, f32) for a in
            (s_snow, s_water, precp_series, tmean_series, lday_series,
             time_series, W1, b1, W2, b2, W3, b3, W4, b4)]
    (s_snow, s_water, precp, tmean, lday, tser,
     W1, b1, W2, b2, W3, b3, W4, b4) = args

    if _compiled is None:
        nc = _build_device()
        _compiled = _make_cached_runner(nc)
    run = _compiled

    wpk = _pack_weights(W1, b1, W2, b2, W3, b3, W4, b4)
    in_maps = []
    for c in range(NCORES):
        bs = slice(c * BPC, (c + 1) * BPC)
        pt = np.empty((2, NS), f32)
        pt[0] = np.ascontiguousarray(precp[bs].T).reshape(NS)
        pt[1] = np.ascontiguousarray(tmean[bs].T).reshape(NS)
        in_maps.append({
            "ptr": pt,
            "trw": pt[1:2].copy(),
            "ldr": np.ascontiguousarray(lday[bs].T).reshape(1, NS),
            "y0d": np.stack([s_snow[bs, 0], s_water[bs, 0]]).copy(),
            "wpk": wpk,
        })
    res = run(in_maps)

    q = np.empty((B, T), f32)
    for c in range(NCORES):
        qc = np.asarray(res[c]["q"]).reshape(T, BPC)
        q[c * BPC:(c + 1) * BPC, :] = qc.T
    return q


# revision 16
# speedup vs baseline: 6.0056x; 1.1154x over previous
#
# nn_ExpHydroM100 kernel for 8 trn2 NeuronCores — fully on-device.
#
# Sharding: data-parallel over the basin axis B (8 basins per core,
# replicated MLP weights), per the sharding hint. Each core:
#   phase 0 (per time-segment): tier prep (midpoint forcings,
#     step(-temp), ln(lday)) on freshly DMA'd forcing slices
#   phase 1: the 2047-step RK4 scan (sequential in time), 8 segments of
#     256 steps, each a For_i hardware loop (5 steps unrolled per iter);
#     the y trajectory spills to a DRAM scratch tensor per segment
#   phase 2: final MLP pass over all [8, 2048] grid points, streaming
#     (p,t) and y back from DRAM in 512-sample chunks
# No collectives; the host only reshapes inputs/outputs.
#
# Layout notes: SBUF APs must start at partition 0/32/64/96 (PSUM is
# exempt) and every SBUF tensor's bytes are charged against all 128
# partitions (~208 KiB budget). So per-segment quantities live as
# free-dim BLOCKS of two small mega-tensors ([2, *] and [1, *]), forcing
# rows (p,t) and live state rows (s0,s1) are separate blocks with layer 1
# split into two PSUM-accumulating matmuls, and the output nonlinearity
# (relu(sinh(x)) == sinh(relu(x)) = (e^u - e^-u)/2, u = relu(o + b4)) is
# assembled in a PSUM scratch tile where partition offsets are legal.
#
import numpy as np

B, T, H = 64, 2048, 64
NCORES = 8
BPC = B // NCORES          # 8 basins per core
NS = T * BPC               # 16384 samples per core
WC = 350                   # packed weight columns
FD = 512                   # phase-2 free-dim chunk (one PSUM bank of fp32)
UNROLL = 4
SEGLEN = 256               # scan steps per segment
NSEG = (T - 1 + SEGLEN - 1) // SEGLEN
L8 = (SEGLEN + 1) * BPC    # seg cols incl. +1 step lookahead

_compiled = None


def _build_device():
    import concourse.bass as bass
    import concourse.bacc as bacc
    import concourse.mybir as mybir
    from concourse.tile import TileContext

    dt = mybir.dt.float32
    AF = mybir.ActivationFunctionType
    OP = mybir.AluOpType

    nc = bacc.Bacc(None, target_bir_lowering=False)
    ptr = nc.declare_dram_parameter("ptr", [2, NS], dt, isOutput=False)
    ldr = nc.declare_dram_parameter("ldr", [1, NS], dt, isOutput=False)
    y0d = nc.declare_dram_parameter("y0d", [2, BPC], dt, isOutput=False)
    wpk = nc.declare_dram_parameter("wpk", [H, WC], dt, isOutput=False)
    qout = nc.declare_dram_parameter("q", [1, NS], dt, isOutput=True)
    ydram = nc.dram_tensor("yscratch", [2, NS], dt, kind="Internal")

    with TileContext(nc) as tc:
        with (
            tc.tile_pool(name="pers", bufs=1) as pers,
            tc.tile_pool(name="wk", bufs=3) as wk,
            tc.tile_pool(name="ps", bufs=2, space="PSUM") as ps,
            tc.tile_pool(name="ps2", bufs=4, space="PSUM") as ps2,
        ):
            # mega2 blocks ([2, L8] each): 0 PT | 1 PTM | 2 YX | 3 YM1
            #   | 4 YM2 | 5 YE
            # mega1 blocks ([1, L8] each): 0 SNT | 1 SNTM | 2 LNX | 3 LNXM
            #   | 4 TRW | 5 LD | 6 S1
            mega2 = pers.tile([2, 6 * L8], dt, tag="mega2")
            mega1 = pers.tile([1, 7 * L8], dt, tag="mega1")
            tw = pers.tile([H, WC], dt, tag="tw")

            def m2(b, c):  # [2, 8] slice of mega2 block b at seg-col c
                if isinstance(c, int):
                    return mega2[0:2, b * L8 + c: b * L8 + c + BPC]
                return mega2[0:2, bass.ds(b * L8 + c, BPC)]

            def m1(b, c):
                if isinstance(c, int):
                    return mega1[0:1, b * L8 + c: b * L8 + c + BPC]
                return mega1[0:1, bass.ds(b * L8 + c, BPC)]

            def blk2(b, c0, n):  # [2, n] block slice (static)
                return mega2[0:2, b * L8 + c0: b * L8 + c0 + n]

            def blk1(b, c0, n):
                return mega1[0:1, b * L8 + c0: b * L8 + c0 + n]

            nc.sync.dma_start(tw[:], wpk[:])

            W2 = tw[:, 0:64]
            W3 = tw[:, 64:128]
            W4q = tw[:, 128:129]
            W4u = tw[:, 129:132]
            W4e = tw[:, 132:134]
            b1 = tw[:, 134:135]
            b2 = tw[:, 135:136]
            b3 = tw[:, 136:137]
            bq = tw[0:1, 137:138]
            W1y = tw[0:2, 138:202]
            W1pt = tw[0:2, 202:266]
            W1ptm = tw[0:2, 266:330]
            oh2 = tw[0:1, 330:332]
            sel_sn = tw[0:1, 332:335]
            sel_g = tw[0:2, 335:338]
            sel_q = tw[0:2, 338:340]
            Ma3 = tw[0:3, 340:342]
            Mb3 = tw[0:3, 342:344]
            Me2 = tw[0:2, 344:346]
            Mbe2 = tw[0:2, 346:348]
            b4a = tw[0:3, 348:349]
            b4e = tw[0:2, 349:350]

            # All engine APs (SBUF and PSUM) must start at partition
            # 0/32/64/96 (matmul OUTPUTS only 0/32/64), so small PSUM tiles
            # share one bank at quadrant offsets: 0:3 pou (o0..o2)
            #   | 32:35 facp [snt,snt,gt0] | 64:66 facq [gt1,gt1];
            # pod (o3+ln(lday), o4) gets its own tile.
            def stage(ptb, ptc, W1p, yb, yc, snb, snc, lnb, lnc):
                ph1 = ps.tile([H, BPC], dt, tag="ph")
                h1 = wk.tile([H, BPC], dt, tag="h1")
                nc.tensor.matmul(ph1[:], W1p, m2(ptb, ptc), start=True, stop=False)
                nc.tensor.matmul(ph1[:], W1y, m2(yb, yc), start=False, stop=True)
                nc.scalar.activation(h1[:], ph1[:], AF.Tanh, bias=b1)
                ph2 = ps.tile([H, BPC], dt, tag="ph")
                h2 = wk.tile([H, BPC], dt, tag="h2")
                nc.tensor.matmul(ph2[:], W2, h1[:], start=True, stop=True)
                nc.scalar.activation(h2[:], ph2[:], AF.Tanh, bias=b2)
                ph3 = ps.tile([H, BPC], dt, tag="ph")
                h3 = wk.tile([H, BPC], dt, tag="h3")
                nc.tensor.matmul(ph3[:], W3, h2[:], start=True, stop=True)
                nc.scalar.activation(h3[:], ph3[:], AF.Tanh, bias=b3)
                pou = ps2.tile([3, BPC], dt, tag="pou", bufs=1)
                pod = ps2.tile([2, BPC], dt, tag="pod", bufs=1)
                facp = ps2.tile([3, BPC], dt, tag="facp", bufs=1)
                facq = ps2.tile([2, BPC], dt, tag="facq", bufs=1)
                pou, pod, facp, facq = pou[:], pod[:], facp[:], facq[:]
                gt2 = wk.tile([2, BPC], dt, tag="gt2")
                nc.scalar.activation(gt2[:], m2(yb, yc), AF.Tanh, scale=5.0)
                nc.tensor.matmul(pod, oh2, m1(lnb, lnc), start=True, stop=False)
                nc.tensor.matmul(pod, W4e, h3[:], start=False, stop=True)
                nc.tensor.matmul(pou, W4u, h3[:], start=True, stop=True)
                nc.tensor.matmul(facp, sel_sn, m1(snb, snc), start=True, stop=False)
                nc.tensor.matmul(facp, sel_g, gt2[:], start=False, stop=True)
                nc.tensor.matmul(facq, sel_q, gt2[:], start=True, stop=True)
                u = wk.tile([3, BPC], dt, tag="u")
                ep = wk.tile([3, BPC], dt, tag="ep")
                en = wk.tile([3, BPC], dt, tag="en")
                sh3 = wk.tile([3, BPC], dt, tag="sh3")
                ee2 = wk.tile([2, BPC], dt, tag="ee2")
                shm3 = wk.tile([3, BPC], dt, tag="shm3")
                eem2 = wk.tile([2, BPC], dt, tag="eem2")
                nc.scalar.activation(u[:], pou, AF.Relu, bias=b4a)
                nc.scalar.activation(ep[:], u[:], AF.Exp)
                nc.vector.reciprocal(en[:], ep[:])
                nc.scalar.activation(ee2[:], pod, AF.Exp, bias=b4e)
                nc.vector.tensor_sub(sh3[:], ep[:], en[:])
                nc.vector.tensor_mul(shm3[:], sh3[:], facp)
                nc.vector.tensor_mul(eem2[:], ee2[:], facq)
                kst = ps2.tile([2, BPC], dt, tag="kst", bufs=2)
                nc.tensor.matmul(kst[:], Me2, ee2[:], start=True, stop=False)
                nc.tensor.matmul(kst[:], Mbe2, eem2[:], start=False, stop=False)
                nc.tensor.matmul(kst[:], Ma3, sh3[:], start=False, stop=False)
                nc.tensor.matmul(kst[:], Mb3, shm3[:], start=False, stop=True)
                return kst

            def step(ci, cip1):
                y = m2(2, ci)
                k1 = stage(0, ci, W1pt, 2, ci, 0, ci, 2, ci)[:]
                nc.vector.scalar_tensor_tensor(
                    m2(3, ci), k1, 0.5, y, OP.mult, OP.add)
                u1 = wk.tile([2, BPC], dt, tag="uc")
                nc.vector.scalar_tensor_tensor(
                    u1[:], k1, 1.0 / 6.0, y, OP.mult, OP.add)
                k2 = stage(1, ci, W1ptm, 3, ci, 1, ci, 3, ci)[:]
                nc.vector.scalar_tensor_tensor(
                    m2(4, ci), k2, 0.5, y, OP.mult, OP.add)
                u2 = wk.tile([2, BPC], dt, tag="uc")
                nc.vector.scalar_tensor_tensor(
                    u2[:], k2, 1.0 / 3.0, u1[:], OP.mult, OP.add)
                k3 = stage(1, ci, W1ptm, 4, ci, 1, ci, 3, ci)[:]
                nc.vector.tensor_add(m2(5, ci), k3, y)
                u3 = wk.tile([2, BPC], dt, tag="uc")
                nc.vector.scalar_tensor_tensor(
                    u3[:], k3, 1.0 / 3.0, u2[:], OP.mult, OP.add)
                k4 = stage(0, cip1, W1pt, 5, ci, 0, cip1, 2, cip1)[:]
                nc.vector.scalar_tensor_tensor(
                    m2(2, cip1), k4, 1.0 / 6.0, u3[:], OP.mult, OP.add)

            for s in range(NSEG):
                seg_start = s * SEGLEN
                nst = min(SEGLEN, (T - 1) - seg_start)
                c8 = seg_start * BPC
                ncols = nst * BPC + BPC  # forcing cols incl. lookahead

                # ---- per-segment phase 0 ----
                nc.sync.dma_start(blk2(0, 0, ncols), ptr[0:2, c8:c8 + ncols])
                nc.sync.dma_start(blk1(4, 0, ncols), ptr[1:2, c8:c8 + ncols])
                nc.sync.dma_start(blk1(5, 0, ncols), ldr[0:1, c8:c8 + ncols])
                if s == 0:
                    nc.sync.dma_start(mega2[0:2, 2 * L8:2 * L8 + BPC], y0d[:])
                else:
                    nc.vector.tensor_copy(m2(2, 0), m2(2, SEGLEN * BPC))
                nm = nst * BPC
                nc.vector.tensor_add(blk2(1, 0, nm), blk2(0, 0, nm),
                                     blk2(0, BPC, nm))
                nc.vector.tensor_add(blk1(6, 0, nm), blk1(4, 0, nm),
                                     blk1(4, BPC, nm))
                nc.scalar.activation(blk1(1, 0, nm), blk1(6, 0, nm),
                                     AF.Tanh, scale=-2.5)
                nc.scalar.activation(blk1(0, 0, ncols), blk1(4, 0, ncols),
                                     AF.Tanh, scale=-5.0)
                nc.vector.tensor_add(blk1(6, 0, nm), blk1(5, 0, nm),
                                     blk1(5, BPC, nm))
                nc.scalar.activation(blk1(3, 0, nm), blk1(6, 0, nm),
                                     AF.Ln, scale=0.5)
                nc.scalar.activation(blk1(2, 0, ncols), blk1(5, 0, ncols),
                                     AF.Ln)

                # ---- scan over this segment ----
                nit = nst // UNROLL
                with tc.For_i(0, nit) as it:
                    base = it * (BPC * UNROLL)
                    for j in range(UNROLL):
                        step(base + BPC * j, base + BPC * (j + 1))
                for i in range(nit * UNROLL, nst):
                    step(i * BPC, (i + 1) * BPC)

                # spill y trajectory for this segment's steps
                nc.sync.dma_start(ydram[0:2, c8:c8 + nm],
                                  blk2(2, 0, nm))
            # final state y(T-1) lives at seg col nst*BPC of the last segment
            lc8 = (NSEG - 1) * SEGLEN * BPC
            lnm = ((T - 1) - (NSEG - 1) * SEGLEN) * BPC
            nc.sync.dma_start(ydram[0:2, lc8 + lnm:lc8 + lnm + BPC],
                              blk2(2, lnm, BPC))

            # ---- phase 2: final MLP pass over all NS samples ----
            for ch in range(NS // FD):
                sl = slice(ch * FD, (ch + 1) * FD)
                p2pt = wk.tile([2, FD], dt, tag="p2pt")
                p2y = wk.tile([2, FD], dt, tag="p2y")
                nc.sync.dma_start(p2pt[:], ptr[0:2, sl])
                nc.sync.dma_start(p2y[:], ydram[0:2, sl])
                pa = ps.tile([H, FD], dt, tag="ph")
                ha = wk.tile([H, FD], dt, tag="h1")
                nc.tensor.matmul(pa[:], W1pt, p2pt[:], start=True, stop=False)
                nc.tensor.matmul(pa[:], W1y, p2y[:], start=False, stop=True)
                nc.scalar.activation(ha[:], pa[:], AF.Tanh, bias=b1)
                pb = ps.tile([H, FD], dt, tag="ph")
                hb = wk.tile([H, FD], dt, tag="h2")
                nc.tensor.matmul(pb[:], W2, ha[:], start=True, stop=True)
                nc.scalar.activation(hb[:], pb[:], AF.Tanh, bias=b2)
                pc = ps.tile([H, FD], dt, tag="ph")
                hc = wk.tile([H, FD], dt, tag="h3")
                nc.tensor.matmul(pc[:], W3, hb[:], start=True, stop=True)
                nc.scalar.activation(hc[:], pc[:], AF.Tanh, bias=b3)
                pq = ps2.tile([1, FD], dt, tag="pod", bufs=1)
                qch = wk.tile([1, FD], dt, tag="qch")
                nc.tensor.matmul(pq[:], W4q, hc[:], start=True, stop=True)
                nc.vector.tensor_scalar_add(qch[:], pq[:], bq)
                nc.sync.dma_start(qout[0:1, sl], qch[:])

    nc.compile()
    return nc


def _pack_weights(W1, b1, W2, b2, W3, b3, W4, b4):
    f32 = np.float32
    wpk = np.zeros((H, WC), f32)
    wpk[:, 0:64] = W2
    wpk[:, 64:128] = W3
    wpk[:, 128] = W4[:, 4]
    wpk[:, 129:134] = W4
    wpk[:, 134] = b1
    wpk[:, 135] = b2
    wpk[:, 136] = b3
    wpk[0, 137] = b4[4]
    wpk[0:2, 138:202] = W1[0:2]
    wpk[0:2, 202:266] = W1[2:4]
    wpk[0:2, 266:330] = 0.5 * W1[2:4]
    wpk[0, 330:332] = np.array([1, 0], f32)            # oh2
    wpk[0, 332:335] = np.array([1, 1, 0], f32)          # sel_sn
    wpk[0:2, 335:338] = np.array([[0, 0, 1], [0, 0, 0]], f32)  # sel_g
    wpk[0:2, 338:340] = np.array([[0, 0], [1, 1]], f32)        # sel_q
    wpk[0:3, 340:342] = np.array([[.25, 0], [0, .5], [-.25, .25]], f32)  # Ma3
    wpk[0:3, 342:344] = np.array([[.25, 0], [0, 0], [-.25, .25]], f32)   # Mb3
    wpk[0:2, 344:346] = np.array([[0, -.5], [0, -.5]], f32)    # Me2
    wpk[0:2, 346:348] = np.array([[0, -.5], [0, -.5]], f32)    # Mbe2
    wpk[0:3, 348] = b4[0:3]
    wpk[0:2, 349] = b4[3:5]
    return wpk


def _make_cached_runner(nc):
    """One-time jax.jit of the bass program (same _bass_exec_p lowering that
    run_bass_kernel_spmd uses under axon) so repeat calls skip re-tracing."""
    import jax
    import numpy as onp
    from jax.sharding import Mesh, PartitionSpec
    from jax.experimental.shard_map import shard_map
    import concourse.mybir as mybir
    from concourse.bass2jax import (
        install_neuronx_cc_hook, _bass_exec_p, partition_id_tensor)
    install_neuronx_cc_hook()

    partition_name = (nc.partition_id_tensor.name
                      if nc.partition_id_tensor else None)
    in_names, out_names, out_avals, zero_outs = [], [], [], []
    for alloc in nc.m.functions[0].allocations:
        if not isinstance(alloc, mybir.MemoryLocationSet):
            continue
        name = alloc.memorylocations[0].name
        if alloc.kind == "ExternalInput":
            if name != partition_name:
                in_names.append(name)
        elif alloc.kind == "ExternalOutput":
            shape = tuple(alloc.tensor_shape)
            dtype = mybir.dt.np(alloc.dtype)
            out_names.append(name)
            out_avals.append(jax.core.ShapedArray(shape, dtype))
            zero_outs.append(onp.zeros(shape, dtype))
    n_params, n_outs = len(in_names), len(out_avals)
    in_names_full = in_names + out_names + (
        [partition_name] if partition_name else [])
    donate = tuple(range(n_params, n_params + n_outs))

    def _body(*xs):
        ops = list(xs)
        if partition_name is not None:
            ops.append(partition_id_tensor())
        return tuple(_bass_exec_p.bind(
            *ops, out_avals=tuple(out_avals), in_names=tuple(in_names_full),
            out_names=tuple(out_names), lowering_input_output_aliases=(),
            sim_require_finite=True, sim_require_nnan=True, nc=nc))

    devices = jax.devices()[:NCORES]
    mesh = Mesh(onp.asarray(devices), ("core",))
    in_specs = (PartitionSpec("core"),) * (n_params + n_outs)
    out_specs = (PartitionSpec("core"),) * n_outs
    sharded = jax.jit(
        shard_map(_body, mesh=mesh, in_specs=in_specs,
                  out_specs=out_specs, check_rep=False),
        donate_argnums=donate, keep_unused=True)

    def run(in_maps):
        per_core = [[onp.asarray(m[nm]) for nm in in_names] for m in in_maps]
        concat_in = [
            onp.concatenate([per_core[c][i] for c in range(NCORES)], axis=0)
            for i in range(n_params)]
        concat_zeros = [
            onp.zeros((NCORES * z.shape[0], *z.shape[1:]), z.dtype)
            for z in zero_outs]
        outs = sharded(*concat_in, *concat_zeros)
        return [{nm: onp.asarray(outs[i]).reshape(
                    NCORES, *out_avals[i].shape)[c]
                 for i, nm in enumerate(out_names)}
                for c in range(NCORES)]

    return run


def kernel(s_snow, s_water, precp_series, tmean_series, lday_series, time_series,
           W1, b1, W2, b2, W3, b3, W4, b4):
    global _compiled
    f32 = np.float32
    args = [np.asarray(a, f32) for a in
            (s_snow, s_water, precp_series, tmean_series, lday_series,
             time_series, W1, b1, W2, b2, W3, b3, W4, b4)]
    (s_snow, s_water, precp, tmean, lday, tser,
     W1, b1, W2, b2, W3, b3, W4, b4) = args

    if _compiled is None:
        nc = _build_device()
        _compiled = _make_cached_runner(nc)
    run = _compiled

    wpk = _pack_weights(W1, b1, W2, b2, W3, b3, W4, b4)
    in_maps = []
    for c in range(NCORES):
        bs = slice(c * BPC, (c + 1) * BPC)
        pt = np.empty((2, NS), f32)
        pt[0] = np.ascontiguousarray(precp[bs].T).reshape(NS)
        pt[1] = np.ascontiguousarray(tmean[bs].T).reshape(NS)
        in_maps.append({
            "ptr": pt,
            "ldr": np.ascontiguousarray(lday[bs].T).reshape(1, NS),
            "y0d": np.stack([s_snow[bs, 0], s_water[bs, 0]]).copy(),
            "wpk": wpk,
        })
    res = run(in_maps)

    q = np.empty((B, T), f32)
    for c in range(NCORES):
        qc = np.asarray(res[c]["q"]).reshape(T, BPC)
        q[c * BPC:(c + 1) * BPC, :] = qc.T
    return q
